# revision 1
# baseline (speedup 1.0000x reference)
"""Contrastive loss kernel for Trainium2 (8 NeuronCores, batch-parallel).

Problem (hardcoded):
  X: (32, 16384, 256) f32   pair embeddings, e_a = X[..., :128], e_b = X[..., 128:]
  y: (32, 128, 128)  i32    adjacency in {0, 1}
  out: (32, 16384)   f32    where(y==1, dist2, relu(1 - dist2))

Sharding: data-parallel over batch, 4 batches per core, no communication.
"""

from contextlib import ExitStack

import numpy as np

import concourse.bass as bass
import concourse.tile as tile
from concourse import bacc, masks, mybir
from concourse.bass_utils import run_bass_kernel_spmd

F32 = mybir.dt.float32
I32 = mybir.dt.int32

B, P, D = 32, 16384, 256
H = D // 2  # 128
ALPHA_MARGIN = 1.0
N_CORES = 8
BPC = B // N_CORES  # batches per core

PART = 128           # SBUF partitions; also pairs per result column
SLOTS = 8            # pair-columns per big tile
TILES = P // (PART * SLOTS)  # big tiles per batch (16)


def build_program(bpc=BPC, slots=SLOTS, tiles=None, pairs=P, passes=1,
                  xbufs=3, dma_split=False):
    """Build the per-core Bass program. Shapes are per-core (full batch dim / 8).

    passes>1 repeats the whole computation (idempotent) — used only for
    marginal-time benchmarking, never for the graded kernel."""
    if tiles is None:
        tiles = pairs // (PART * slots)
    assert tiles * slots * PART == pairs
    ncols = tiles * slots  # result columns per batch (pairs // 128)

    nc = bacc.Bacc("TRN2", target_bir_lowering=False, debug=False,
                   num_devices=N_CORES)
    X = nc.dram_tensor("X", [bpc, pairs, D], F32, kind="ExternalInput").ap()
    Y = nc.dram_tensor("y", [bpc, pairs], I32, kind="ExternalInput").ap()
    O = nc.dram_tensor("out", [bpc, pairs], F32, kind="ExternalOutput").ap()

    with tile.TileContext(nc) as tc, ExitStack() as ctx:
        xpool = ctx.enter_context(tc.tile_pool(name="x", bufs=xbufs))
        dpool = ctx.enter_context(tc.tile_pool(name="diff", bufs=3))
        rpool = ctx.enter_context(tc.tile_pool(name="res", bufs=2))
        ppool = ctx.enter_context(tc.tile_pool(name="psum", bufs=2, space="PSUM"))
        spool = ctx.enter_context(tc.tile_pool(name="small", bufs=2))
        cpool = ctx.enter_context(tc.tile_pool(name="const", bufs=1))

        ident = cpool.tile([PART, PART], F32)
        masks.make_identity(nc, ident[:])
        ones = cpool.tile([PART, 1], F32)
        nc.gpsimd.memset(ones[:], 1.0)

        for b in [b for _ in range(passes) for b in range(bpc)]:
            # pair index = t*128 + p  ->  [p, t, f] view of X[b]
            Xb = X[b].rearrange("(t p) f -> p t f", p=PART)
            res = rpool.tile([PART, ncols], F32)
            for g in range(tiles):
                xt = xpool.tile([PART, slots, D], F32)
                dma_eng = nc.scalar if (dma_split and g % 2) else nc.sync
                dma_eng.dma_start(xt[:], Xb[:, g * slots:(g + 1) * slots, :])
                dft = dpool.tile([PART, slots, H], F32)
                nc.vector.tensor_sub(dft[:], xt[:, :, 0:H], xt[:, :, H:D])
                for j in range(slots):
                    c = g * slots + j
                    # DVE: out = diff * diff, accum_out = sum -> dist2.
                    # All-DVE beats splitting with ACT (modeled 197.5 vs
                    # 198.0/257.2 us): DVE stays 70 us under the DMA span
                    # while ACT's per-op overhead is 2.6x DVE's.
                    nc.vector.scalar_tensor_tensor(
                        out=dft[:, j, :], in0=dft[:, j, :], scalar=0.0,
                        in1=dft[:, j, :],
                        op0=mybir.AluOpType.bypass, op1=mybir.AluOpType.mult,
                        accum_out=res[:, c:c + 1],
                    )

            # res[p, t] = dist2(pair t*128+p); transpose so partition = t
            pres = ppool.tile([ncols, PART], F32)
            nc.tensor.transpose(pres[:], res[:], ident[:])

            yt = spool.tile([ncols, PART], I32)
            nc.sync.dma_start(yt[:], Y[b].rearrange("(t p) -> t p", p=PART))

            # outt = relu(margin - dist2); then overwrite y==1 entries with dist2
            outt = spool.tile([ncols, PART], F32)
            nc.scalar.activation(
                outt[:], pres[:], mybir.ActivationFunctionType.Relu,
                scale=-1.0, bias=ones[0:ncols, 0:1],
            )
            nc.vector.copy_predicated(outt[:], yt[:], pres[:])

            nc.sync.dma_start(O[b].rearrange("(t p) -> t p", p=PART), outt[:])

    nc.compile()
    return nc


_PROGRAM_CACHE = {}


def _get_program():
    if "nc" not in _PROGRAM_CACHE:
        _PROGRAM_CACHE["nc"] = build_program()
    return _PROGRAM_CACHE["nc"]


def kernel(X, y):
    import os
    if os.environ.get("BASS_TRACE"):
        # The axon NTFF trace path needs antenv.axon_hooks, which some
        # images lack; fall back to untraced execution rather than crash.
        try:
            import antenv.axon_hooks  # noqa: F401
        except ImportError:
            os.environ["BASS_NEVER_TRACE"] = "1"

    X = np.asarray(X, dtype=np.float32)
    y = np.asarray(y, dtype=np.int32).reshape(B, P)
    assert X.shape == (B, P, D)

    nc = _get_program()
    in_maps = [
        {"X": np.ascontiguousarray(X[c * BPC:(c + 1) * BPC]),
         "y": np.ascontiguousarray(y[c * BPC:(c + 1) * BPC])}
        for c in range(N_CORES)
    ]
    # The axon-tunneled devices occasionally come up wedged from a prior
    # session (NRT_EXEC_UNIT_UNRECOVERABLE); a backend reset + retry clears it.
    last_err = None
    for attempt in range(3):
        try:
            res = run_bass_kernel_spmd(nc, in_maps, list(range(N_CORES)))
            break
        except Exception as e:  # transient device/tunnel failures
            last_err = e
            import time

            import jax
            try:
                jax.clear_caches()
            except Exception:
                pass
            try:
                jax._src.api.clear_backends()
            except Exception:
                pass
            time.sleep(5.0 * (attempt + 1))
    else:
        raise last_err
    out = np.concatenate([res.results[c]["out"] for c in range(N_CORES)], axis=0)
    return out.astype(np.float32)



# revision 5
# speedup vs baseline: 311.9687x; 311.9687x over previous
"""Contrastive loss kernel for Trainium2 (8 NeuronCores, batch-parallel).

Problem (hardcoded):
  X: (32, 16384, 256) f32   pair embeddings, e_a = X[..., :128], e_b = X[..., 128:]
  y: (32, 128, 128)  i32    adjacency in {0, 1}
  out: (32, 16384)   f32    where(y==1, dist2, relu(1 - dist2))

Sharding: data-parallel over batch, 4 batches per core, no communication.

Layout: the per-core (4, 16384, 256) X block is reshaped host-side to
(128, 512, 256) so each SBUF partition owns a contiguous 512 KiB row range
of HBM — every DMA is partition-contiguous (8 KiB per partition per tile)
and the dist2 accumulator lands directly in output layout: no transpose,
no PSUM, and the X stream on the sync queue never waits on the finalize
chain (y load, relu, predicated merge, stores all live on ACT/DVE).

Schedule details (HW-measured against alternatives):
  - 1 MiB X tiles (slots=8) from a single queue beat 2/4 MiB tiles and
    dual-queue issue; xbufs=5 covers DMA latency (3 is too few).
  - finalize for chunk k is emitted after the first tile of chunk k+1 so
    DVE's in-order wait queue never parks at copy_predicated.
  - 2 of 8 squared-sum columns per tile run on ACT (Square + accum_out),
    trimming DVE's per-tile backlog (~26 us of DVE busy moved to idle ACT).
"""

from contextlib import ExitStack

import numpy as np

import concourse.bass as bass
import concourse.tile as tile
from concourse import bacc, mybir
from concourse.bass_utils import run_bass_kernel_spmd

F32 = mybir.dt.float32
I32 = mybir.dt.int32

B, P, D = 32, 16384, 256
H = D // 2  # 128
ALPHA_MARGIN = 1.0
N_CORES = 8
BPC = B // N_CORES          # batches per core
ROWS = BPC * P // 128       # 512 pair-rows per partition

PART = 128
SLOTS = 8
CHUNK_TILES = 8


def build_program(slots=SLOTS, chunk_tiles=CHUNK_TILES, xbufs=5,
                  act_slots=2, fin_defer=True, loop_passes=0, passes=1,
                  plan=None, dma_act_every=0):
    """Per-core Bass program on the reshaped (128, ROWS, 256) layout.

    plan: optional list of chunks, each a list of per-tile slot counts
    (overrides slots/chunk_tiles). loop_passes>0 wraps the body in a
    tc.For_i hardware loop; passes>1 unrolls it. Both are benchmarking
    aids only — the graded kernel uses the single-pass default."""
    if plan is None:
        tiles = ROWS // slots
        assert tiles % chunk_tiles == 0
        plan = [[slots] * chunk_tiles for _ in range(tiles // chunk_tiles)]
    assert sum(s for ch in plan for s in ch) == ROWS

    nc = bacc.Bacc("TRN2", target_bir_lowering=False, debug=False,
                   num_devices=N_CORES)
    X = nc.dram_tensor("X", [PART, ROWS, D], F32, kind="ExternalInput").ap()
    Y = nc.dram_tensor("y", [PART, ROWS], I32, kind="ExternalInput").ap()
    O = nc.dram_tensor("out", [PART, ROWS], F32, kind="ExternalOutput").ap()

    with tile.TileContext(nc) as tc, ExitStack() as ctx:
        xpool = ctx.enter_context(tc.tile_pool(name="x", bufs=xbufs))
        dpool = ctx.enter_context(tc.tile_pool(name="diff", bufs=3))
        rpool = ctx.enter_context(tc.tile_pool(name="res", bufs=2))
        spool = ctx.enter_context(tc.tile_pool(name="small", bufs=2))
        cpool = ctx.enter_context(tc.tile_pool(name="const", bufs=1))

        ones = cpool.tile([PART, 1], F32)
        nc.gpsimd.memset(ones[:], 1.0)

        def fin(col0, ccols, res, yt):
            # outt = relu(margin - dist2); overwrite y==1 entries with dist2
            outt = spool.tile([PART, ccols], F32)
            nc.scalar.activation(
                outt[:], res[:], mybir.ActivationFunctionType.Relu,
                scale=-1.0, bias=ones[:, 0:1],
            )
            nc.vector.copy_predicated(
                outt[:], yt[:, col0:col0 + ccols], res[:])
            nc.scalar.dma_start(O[:, col0:col0 + ccols], outt[:])

        def body():
            yt = cpool.tile([PART, ROWS], I32)
            nc.scalar.dma_start(yt[:], Y[:, :])

            pending = None
            row = 0
            gidx = [0]
            for chunk in plan:
                ccols = sum(chunk)
                col0 = row
                res = rpool.tile([PART, ccols], F32)
                c = 0
                for gg, ts in enumerate(chunk):
                    xt = xpool.tile([PART, ts, D], F32)
                    eng = (nc.scalar if dma_act_every and
                           (gidx[0] % dma_act_every == dma_act_every - 1)
                           else nc.sync)
                    eng.dma_start(xt[:], X[:, row:row + ts, :])
                    gidx[0] += 1
                    row += ts
                    dft = dpool.tile([PART, ts, H], F32)
                    nc.vector.tensor_sub(dft[:], xt[:, :, 0:H], xt[:, :, H:D])
                    for j in range(ts):
                        if j < ts - act_slots:
                            # DVE: out = diff * diff, accum_out = sum -> dist2
                            nc.vector.scalar_tensor_tensor(
                                out=dft[:, j, :], in0=dft[:, j, :], scalar=0.0,
                                in1=dft[:, j, :],
                                op0=mybir.AluOpType.bypass,
                                op1=mybir.AluOpType.mult,
                                accum_out=res[:, c:c + 1],
                            )
                        else:
                            nc.scalar.activation(
                                dft[:, j, :], dft[:, j, :],
                                mybir.ActivationFunctionType.Square,
                                accum_out=res[:, c:c + 1],
                            )
                        c += 1
                    if gg == 0 and pending is not None and fin_defer:
                        fin(*pending, yt)
                        pending = None
                if fin_defer:
                    pending = (col0, ccols, res)
                else:
                    fin(col0, ccols, res, yt)
            if pending is not None:
                fin(*pending, yt)

        if loop_passes:
            with tc.For_i(0, loop_passes):
                body()
        else:
            for _ in range(passes):
                body()

    nc.compile()
    return nc


_PROGRAM_CACHE = {}


def _get_program():
    if "nc" not in _PROGRAM_CACHE:
        _PROGRAM_CACHE["nc"] = build_program()
    return _PROGRAM_CACHE["nc"]


def kernel(X, y):
    import os
    if os.environ.get("BASS_TRACE"):
        # The axon NTFF trace path needs antenv.axon_hooks, which some
        # images lack; fall back to untraced execution rather than crash.
        try:
            import antenv.axon_hooks  # noqa: F401
        except ImportError:
            os.environ["BASS_NEVER_TRACE"] = "1"

    X = np.asarray(X, dtype=np.float32)
    y = np.asarray(y, dtype=np.int32).reshape(B, P)
    assert X.shape == (B, P, D)

    nc = _get_program()
    in_maps = [
        {"X": np.ascontiguousarray(
            X[c * BPC:(c + 1) * BPC]).reshape(PART, ROWS, D),
         "y": np.ascontiguousarray(
            y[c * BPC:(c + 1) * BPC]).reshape(PART, ROWS)}
        for c in range(N_CORES)
    ]
    # The axon-tunneled devices occasionally come up wedged from a prior
    # session (NRT_EXEC_UNIT_UNRECOVERABLE); a backend reset + retry clears it.
    last_err = None
    for attempt in range(3):
        try:
            res = run_bass_kernel_spmd(nc, in_maps, list(range(N_CORES)))
            break
        except Exception as e:  # transient device/tunnel failures
            last_err = e
            import time

            import jax
            try:
                jax.clear_caches()
            except Exception:
                pass
            try:
                jax._src.api.clear_backends()
            except Exception:
                pass
            time.sleep(5.0 * (attempt + 1))
    else:
        raise last_err
    out = np.concatenate(
        [res.results[c]["out"].reshape(BPC, P) for c in range(N_CORES)], axis=0)
    return out.astype(np.float32)
